# revision 1
# baseline (speedup 1.0000x reference)
"""Trainium2 Bass kernel for the BiaffineLayer problem.

Math (per batch b):
  out[l, m, c] = x1[l] @ W1[c] + x2[m] @ W2[c]
              + sum_h x1[l,h] * x2[m,h] * W3[c,h]
              + sum_h |x1[l,h] - x2[m,h]| * W4[c,h] + bias[c]
  shapes: x1, x2 [2, 512, 128]; W [25, 512]; bias [25]; out [2, 512, 512, 25]

Sharding: 8 cores = 2 batches x 4 m-blocks of 128 columns. Each core gets the
full x1[b] (transposed to [h, l]) and its x2[b, m0:m0+128] block (transposed),
W/bias replicated, and produces out[b, :, m0:m0+128, :] = [512, 128, 25].

Per-core dataflow:
  The abs term uses |d| = 2*relu(d) - d: the -d part is rank-structured
  (x1@W4T folds into the V3' moving operand, x2@W4T folds into T2B), so only
  relu(x1 - x2[m]) is pairwise.
  - D_m[h, l] = relu(x1t - x2t[:, m]) in bf16 via one fused tensor_scalar
    (op0=subtract, op1=max 0) on DVE, or Relu-activation with bias on ACT.
  - t4 pairwise part: matmul with D_m l-chunk as the stationary operand (bf16)
    and 2*W4T [h, 25] moving; accumulates into PSUM [l-chunk, (m, c)].
  - t3+t1-t4a: one bf16 matmul per chunk: x1t l-chunk stationary,
    V3'[h, (m, c)] = x2t[h,m]*W3T[h,c] + (W1T - W4T)[h,c] moving.
  - t2+bias+t4b: computed once into a [m, 25] PSUM tile (x2t @ (W2+W4)T plus a
    K=1 ones-matmul adding bias), collapsed to one DRAM row, reloaded as a
    bf16 [1, m*c] row, and added into every chunk's PSUM via a K=1
    ones-matmul; PSUM->SBUF copy on the scalar engine, then DMA out.

  Host-side wmov packing: [W1T-W4T | W2T+W4T | W3T | 2*W4T], each [128, 25].
"""

import sys

sys.path.insert(0, "/opt/trn_rl_repo")

from contextlib import ExitStack

import numpy as np

import concourse.bass as bass
import concourse.tile as tile
from concourse import bacc, bass_utils, mybir

F32 = mybir.dt.float32
F32R = mybir.dt.float32r
BF16 = mybir.dt.bfloat16

B, L, H, C = 2, 512, 128, 25
MB = 128          # m-block per core
N_CORES = 8
MSUB = 16         # m's per psum chunk
N_MS = MB // MSUB  # 8 chunks over the m-block
LCHUNK = 128
N_LC = L // LCHUNK  # 4 l-chunks
CHUNK_F = MSUB * C  # 400 psum free columns per chunk
N_DVE_D = 12      # of each 16 D-tiles: this many on DVE, rest on ACT


def build_kernel(nc: bass.Bass, repeat: int = 1):
    x1t = nc.dram_tensor("x1t", (H, L), F32, kind="ExternalInput").ap()
    x2t = nc.dram_tensor("x2t", (H, MB), F32, kind="ExternalInput").ap()
    wmov = nc.dram_tensor("wmov", (H, 4 * C), F32, kind="ExternalInput").ap()
    brow = nc.dram_tensor("brow", (1, C), F32, kind="ExternalInput").ap()
    out = nc.dram_tensor("out", (L, MB * C), F32, kind="ExternalOutput").ap()

    with tile.TileContext(nc) as tc, ExitStack() as ctx:
      const = ctx.enter_context(tc.tile_pool(name="const", bufs=1))
      dpool = ctx.enter_context(tc.tile_pool(name="dpool", bufs=48))
      opool = ctx.enter_context(tc.tile_pool(name="opool", bufs=10))
      psum = ctx.enter_context(tc.tile_pool(name="psum", bufs=7, space="PSUM"))
      psum_t2 = ctx.enter_context(tc.tile_pool(name="psum_t2", bufs=1, space="PSUM"))
      dram = ctx.enter_context(tc.tile_pool(name="dram", bufs=1, space="DRAM"))
      for _rep in range(repeat):
        # ---- constant loads ----
        x1t_f = const.tile([H, L], F32)
        nc.sync.dma_start(x1t_f[:], x1t[:])
        x1t_bf = const.tile([H, L], BF16)
        nc.vector.tensor_copy(x1t_bf[:], x1t_f[:])
        x2t_f = const.tile([H, MB], F32)
        nc.sync.dma_start(x2t_f[:], x2t[:])
        wmov_f = const.tile([H, 4 * C], F32)
        nc.sync.dma_start(wmov_f[:], wmov[:])
        w4t_bf = const.tile([H, C], BF16)
        nc.vector.tensor_copy(w4t_bf[:], wmov_f[:, 3 * C : 4 * C])
        w2t_bf = const.tile([H, C], BF16)
        nc.vector.tensor_copy(w2t_bf[:], wmov_f[:, C : 2 * C])
        x2t_bf = const.tile([H, MB], BF16)
        nc.vector.tensor_copy(x2t_bf[:], x2t_f[:])
        brow_bf = const.tile([1, C], BF16)
        nc.gpsimd.dma_start(brow_bf[:], brow[:])
        ones_bf = const.tile([1, MB], BF16)
        nc.vector.memset(ones_bf[:], 1.0)
        negx2_f = const.tile([H, MB], F32)
        nc.vector.tensor_scalar_mul(negx2_f[:], x2t_f[:], -1.0)

        # ---- T2B = (t2[m, c] + bias[c]) broadcast to all partitions ----
        ps_t2 = psum_t2.tile([MB, C], F32)
        nc.tensor.matmul(ps_t2[:], x2t_bf[:], w2t_bf[:],
                         start=True, stop=False, skip_group_check=True)
        nc.tensor.matmul(ps_t2[:], ones_bf[:], brow_bf[:],
                         start=False, stop=True, skip_group_check=True)
        t2small = const.tile([MB, C], F32)
        nc.scalar.copy(t2small[:], ps_t2[:])
        t2_dram = dram.tile([1, MB * C], F32)
        nc.sync.dma_start(t2_dram[:].rearrange("o (m c) -> (o m) c", c=C), t2small[:])
        t2row_bf = const.tile([1, MB * C], BF16)
        nc.gpsimd.dma_start(t2row_bf[:], t2_dram[:])

        # ---- V3'[h, (m, c)] = x2t[h,m] * W3T[h,c] + W1T[h,c]  (bf16) ----
        v3a = const.tile([H, MB * C], F32)
        v3 = const.tile([H, MB * C], BF16)
        VS = 2 * MSUB  # V3 slice width in m's (two m-subs)
        w3_bc = wmov_f[:, 2 * C : 3 * C].unsqueeze(1).broadcast_to([H, VS, C])
        w1_bc = wmov_f[:, 0:C].unsqueeze(1).broadcast_to([H, VS, C])

        def v3_prep(vh):
            sl = slice(vh * VS * C, (vh + 1) * VS * C)
            x2_bc = (x2t_f[:, vh * VS : (vh + 1) * VS]
                     .unsqueeze(2).broadcast_to([H, VS, C]))
            v3a_3d = v3a[:, sl].rearrange("h (m c) -> h m c", c=C)
            nc.vector.tensor_tensor(v3a_3d, x2_bc, w3_bc, op=mybir.AluOpType.mult)
            nc.vector.tensor_tensor(v3[:, sl].rearrange("h (m c) -> h m c", c=C),
                                    v3a_3d, w1_bc, op=mybir.AluOpType.add)

        v3_prep(0)

        # ---- main loop ----
        for ms in range(N_MS):
            if ms % 2 == 0 and ms + 2 < N_MS:
                v3_prep(ms // 2 + 1)
            dts = []
            for j in range(MSUB):
                m = ms * MSUB + j
                dt_ = dpool.tile([H, L], BF16, tag="d")
                n_dve = MSUB if ms >= N_MS - 2 else N_DVE_D
                if j < n_dve:
                    nc.vector.tensor_scalar(
                        dt_[:], x1t_bf[:], x2t_f[:, m : m + 1], 0.0,
                        op0=mybir.AluOpType.subtract, op1=mybir.AluOpType.max)
                else:
                    nc.scalar.activation(
                        dt_[:], x1t_bf[:], mybir.ActivationFunctionType.Relu,
                        bias=negx2_f[:, m : m + 1], scale=1.0)
                dts.append(dt_)
            for lc in range(N_LC):
                ps = psum.tile([LCHUNK, CHUNK_F], F32)
                nc.tensor.matmul(
                    ps[:],
                    x1t_bf[:, lc * LCHUNK : (lc + 1) * LCHUNK],
                    v3[:, ms * CHUNK_F : (ms + 1) * CHUNK_F],
                    start=True, stop=False, skip_group_check=True)
                for j in range(MSUB):
                    nc.tensor.matmul(
                        ps[:, j * C : (j + 1) * C],
                        dts[j][:, lc * LCHUNK : (lc + 1) * LCHUNK],
                        w4t_bf[:],
                        start=False, stop=False, skip_group_check=True)
                nc.tensor.matmul(
                    ps[:], ones_bf[:],
                    t2row_bf[:, ms * CHUNK_F : (ms + 1) * CHUNK_F],
                    start=False, stop=True, skip_group_check=True)
                o_sb = opool.tile([LCHUNK, CHUNK_F], F32)
                if ms >= N_MS - 1:
                    nc.vector.tensor_copy(o_sb[:], ps[:])
                else:
                    nc.scalar.copy(o_sb[:], ps[:])
                nc.sync.dma_start(
                    out[lc * LCHUNK : (lc + 1) * LCHUNK,
                        ms * CHUNK_F : (ms + 1) * CHUNK_F],
                    o_sb[:])
    return nc


_COMPILED = {}


def _get_compiled():
    if "nc" not in _COMPILED:
        nc = bacc.Bacc("TRN2", target_bir_lowering=False, debug=False,
                       num_devices=N_CORES)
        build_kernel(nc)
        nc.compile()
        _COMPILED["nc"] = nc
    return _COMPILED["nc"]


def make_in_maps(x1, x2, W, b):
    W1, W2, W3, W4 = (W[:, 0:H], W[:, H : 2 * H], W[:, 2 * H : 3 * H],
                      W[:, 3 * H : 4 * H])
    wmov = np.ascontiguousarray(
        np.concatenate([(W1 - W4).T, (W2 + W4).T, W3.T, (2.0 * W4).T], axis=1),
        dtype=np.float32)
    brow = np.ascontiguousarray(b.reshape(1, C), dtype=np.float32)
    in_maps = []
    for cid in range(N_CORES):
        bb, mblk = cid // 4, cid % 4
        m0 = mblk * MB
        in_maps.append({
            "x1t": np.ascontiguousarray(x1[bb].T, dtype=np.float32),
            "x2t": np.ascontiguousarray(x2[bb, m0 : m0 + MB].T, dtype=np.float32),
            "wmov": wmov,
            "brow": brow,
        })
    return in_maps


def run_on_device(x1, x2, W, b, trace=False, trace_kwargs=None):
    nc = _get_compiled()
    in_maps = make_in_maps(x1, x2, W, b)
    res = bass_utils.run_bass_kernel_spmd(
        nc, in_maps, core_ids=list(range(N_CORES)), trace=trace,
        **(trace_kwargs or {}))
    full = np.empty((B, L, L, C), dtype=np.float32)
    for cid in range(N_CORES):
        bb, mblk = cid // 4, cid % 4
        m0 = mblk * MB
        full[bb, :, m0 : m0 + MB, :] = res.results[cid]["out"].reshape(L, MB, C)
    return full, res


def kernel(x1, x2, W, b):
    x1 = np.asarray(x1, dtype=np.float32)
    x2 = np.asarray(x2, dtype=np.float32)
    W = np.asarray(W, dtype=np.float32)
    b = np.asarray(b, dtype=np.float32)
    full, _ = run_on_device(x1, x2, W, b, trace=False)
    return full

